# revision 1
# baseline (speedup 1.0000x reference)
"""GAT layer (8-head attention + 4-hop propagation + LayerNorm) on 8 TRN2 cores.

Sharding: data-parallel over batch B=8 — core b computes batch b entirely
(no collectives). Per-core program (all f32 I/O, E/z dtype configurable):

  qT/kT = Wq/Wk @ x^T + b      [512, 1024]  (hidden on partitions)
  v01   = 0.1*(x @ Wv^T + bv)  [1024, 512]  (nodes on partitions)
  per head h (64-dim slices of qT/kT/v01):
    E[m, n] = exp(k_h[m]·q_h[n]) * mask[m, n]          (scores transposed)
    D[n]    = sum_m E[m, n]      (ones-column matmul, fused with hop 1)
    z1 = (E.T @ v01_h) * (9/D)  + v01_h                (= 0.9*P@v + 0.1*v)
    z_{t+1} = (E.T @ z_t) * (0.9/D) + v01_h            (hops 2..4)
    y[:, h*64:+64] = z4
  out = LayerNorm(y + x) * gamma + beta

The softmax max-subtraction is skipped: scores are bounded (~22) so exp()
cannot overflow fp32, and exp(MASK)=0 exactly matches the reference's
masked softmax (verified rel err 5.5e-6 vs jax reference in f32).
"""

import numpy as np

import concourse.bass as bass
import concourse.mybir as mybir
import concourse.tile as tile
from concourse import bacc
from concourse.bass_utils import run_bass_kernel_spmd

B, N, H = 8, 1024, 512
NHEADS, U = 8, 64
P = 128
NT = N // P          # 8 node tiles
KT = H // P          # 4 hidden tiles
ALPHA = 0.1
LN_EPS = 1e-5
F32 = mybir.dt.float32

# E/z dtype for the propagation path: "float32" (rel err ~6e-6) or
# "bfloat16" (rel err ~1.2e-3, ~2x faster weight loads on the hop matmuls).
E_DTYPE_NAME = "bfloat16"

_BUILD_CACHE = {}


def build_nc(edt_name: str, apply_affine: bool):
    key = (edt_name, apply_affine)
    if key in _BUILD_CACHE:
        return _BUILD_CACHE[key]

    EDT = getattr(mybir.dt, edt_name)
    ZDT = EDT
    nc = bacc.Bacc(None, target_bir_lowering=False)

    xT_d = nc.dram_tensor("xT", [H, N], F32, kind="ExternalInput")
    xn_d = nc.dram_tensor("xn", [N, H], F32, kind="ExternalInput")
    maskT_d = nc.dram_tensor("maskT", [N, N], EDT, kind="ExternalInput")
    wq_d = nc.dram_tensor("wqT", [H, H], F32, kind="ExternalInput")
    wk_d = nc.dram_tensor("wkT", [H, H], F32, kind="ExternalInput")
    wv_d = nc.dram_tensor("wvT01", [H, H], F32, kind="ExternalInput")
    bq_d = nc.dram_tensor("bq", [H], F32, kind="ExternalInput")
    bk_d = nc.dram_tensor("bk", [H], F32, kind="ExternalInput")
    bv_d = nc.dram_tensor("bv01", [H], F32, kind="ExternalInput")
    if apply_affine:
        gam_d = nc.dram_tensor("gammar", [P, H], F32, kind="ExternalInput")
        bet_d = nc.dram_tensor("betar", [P, H], F32, kind="ExternalInput")
    out_d = nc.dram_tensor("out", [N, H], F32, kind="ExternalOutput")

    with tile.TileContext(nc) as tc:
        with tc.tile_pool(name="const", bufs=1) as cpool, \
             tc.tile_pool(name="big", bufs=1) as bpool, \
             tc.tile_pool(name="epool", bufs=2 if edt_name == "bfloat16" else 1) as epool, \
             tc.tile_pool(name="zpool", bufs=3) as zpool, \
             tc.tile_pool(name="tpool", bufs=2) as tpool, \
             tc.tile_pool(name="spool", bufs=4) as spool, \
             tc.tile_pool(name="ps512", bufs=2, space="PSUM") as ps512, \
             tc.tile_pool(name="scps", bufs=2, space="PSUM") as scps, \
             tc.tile_pool(name="dps", bufs=2, space="PSUM") as dpsp:

            # ---- persistent SBUF residents ----
            # Loads spread over the sync/gpsimd/scalar/vector DMA queues so
            # the PE-critical tensors (xT + weights) land first in parallel.
            bq_sb = cpool.tile([P, KT], F32)
            nc.gpsimd.dma_start(bq_sb[:], bq_d[:].rearrange("(t p) -> p t", p=P))
            bk_sb = cpool.tile([P, KT], F32)
            nc.scalar.dma_start(bk_sb[:], bk_d[:].rearrange("(t p) -> p t", p=P))
            # bv01 as a [1, 512] row for the rank-1 bias matmul
            bvrow_sb = cpool.tile([P, H], F32, tag="bvrow")
            nc.scalar.dma_start(bvrow_sb[:1, :], bv_d[:].rearrange("(a h) -> a h", a=1))
            maskT_sb = cpool.tile([P, NT, N], EDT)
            xn_sb = cpool.tile([P, NT, H], F32)
            ones_row = cpool.tile([P, P], F32, tag="onesrow")
            nc.vector.memset(ones_row[:1, :], 1.0)
            eps_sb = cpool.tile([P, 1], F32, tag="eps")
            nc.vector.memset(eps_sb[:], LN_EPS)
            if apply_affine:
                gam_sb = cpool.tile([P, H], F32, tag="gam")
                bet_sb = cpool.tile([P, H], F32, tag="bet")

            qT_sb = bpool.tile([P, KT, N], F32, tag="qT")
            kT_sb = bpool.tile([P, KT, N], F32, tag="kT")
            # v01 per-head blocks of 64 values + a trailing 1.0 column; the
            # ones column rides hop 1's moving operand to produce D in PSUM.
            v01_sb = bpool.tile([P, NT, NHEADS, U + 1], ZDT, tag="v01")
            nc.vector.memset(v01_sb[:, :, :, U:U + 1], 1.0)
            y_sb = bpool.tile([P, NT, H], F32, tag="y")

            # ---- phase 1: projections (inside a scope so xT/W free early) ----
            with tc.tile_pool(name="ph1", bufs=1) as p1:
                xT_sb = p1.tile([P, KT, N], F32, tag="xT")
                nc.sync.dma_start(xT_sb[:], xT_d[:, :].rearrange("(t p) n -> p t n", p=P))
                wq_sb = p1.tile([P, KT, H], F32, tag="wq")
                nc.gpsimd.dma_start(wq_sb[:], wq_d[:, :].rearrange("(t p) i -> p t i", p=P))
                wk_sb = p1.tile([P, KT, H], F32, tag="wk")
                nc.scalar.dma_start(wk_sb[:], wk_d[:, :].rearrange("(t p) i -> p t i", p=P))
                wv_sb = p1.tile([P, KT, H], F32, tag="wv")
                nc.scalar.dma_start(wv_sb[:], wv_d[:, :].rearrange("(t p) i -> p t i", p=P))
                # big non-critical loads queue up behind the critical ones
                nc.sync.dma_start(maskT_sb[:], maskT_d[:, :].rearrange("(t p) n -> p t n", p=P))
                nc.gpsimd.dma_start(xn_sb[:], xn_d[:, :].rearrange("(t p) h -> p t h", p=P))
                if apply_affine:
                    nc.gpsimd.dma_start(gam_sb[:], gam_d[:, :])
                    nc.gpsimd.dma_start(bet_sb[:], bet_d[:, :])

                # qT[i, n] = sum_k WqT[k, i] xT[k, n] + bq[i]
                for w_sb, b_sb, dst in ((wq_sb, bq_sb, qT_sb), (wk_sb, bk_sb, kT_sb)):
                    for it in range(KT):
                        for ncx in range(2):
                            ps = ps512.tile([P, 512], F32, tag="ps512")
                            for kt in range(KT):
                                nc.tensor.matmul(
                                    ps[:],
                                    w_sb[:, kt, it * P:(it + 1) * P],
                                    xT_sb[:, kt, ncx * 512:(ncx + 1) * 512],
                                    start=(kt == 0), stop=(kt == KT - 1),
                                )
                            nc.vector.tensor_scalar_add(
                                dst[:, it, ncx * 512:(ncx + 1) * 512], ps[:],
                                b_sb[:, it:it + 1],
                            )

                # v01[node, j] = sum_k xT[k, node] WvT01[k, j] + bv01[j]
                for nt in range(NT):
                    ps = ps512.tile([P, 512], F32, tag="ps512")
                    nc.tensor.matmul(
                        ps[:], ones_row[:1, :P], bvrow_sb[:1, :],
                        start=True, stop=False,
                    )
                    for kt in range(KT):
                        nc.tensor.matmul(
                            ps[:],
                            xT_sb[:, kt, nt * P:(nt + 1) * P],
                            wv_sb[:, kt, :],
                            start=False, stop=(kt == KT - 1),
                        )
                    nc.scalar.activation(
                        v01_sb[:, nt, :, 0:U],
                        ps[:].rearrange("p (h u) -> p h u", u=U),
                        mybir.ActivationFunctionType.Copy,
                    )

            # ---- phase 2: per-head attention + propagation ----
            for h in range(NHEADS):
                pt, po = h // 2, (h % 2) * U
                kh = kT_sb[po:po + U, pt, :]   # [64, 1024] (d on partitions)
                qh = qT_sb[po:po + U, pt, :]

                e_sb = epool.tile([P, NT, N], EDT, tag="E")
                for mt in range(NT):
                    sps = scps.tile([P, N], F32, tag="scps")
                    for ncx in range(2):
                        nc.tensor.matmul(
                            sps[:, ncx * 512:(ncx + 1) * 512],
                            kh[:, mt * P:(mt + 1) * P],
                            qh[:, ncx * 512:(ncx + 1) * 512],
                            start=True, stop=True,
                        )
                    nc.scalar.activation(
                        e_sb[:, mt, :], sps[:], mybir.ActivationFunctionType.Exp,
                    )
                    nc.vector.tensor_tensor(
                        e_sb[:, mt, :], e_sb[:, mt, :], maskT_sb[:, mt, :],
                        mybir.AluOpType.mult,
                    )

                w0 = v01_sb[:, :, h, 0:U]  # [128, 8, 64]
                rd09 = spool.tile([P, NT], F32, tag="rd09")
                rd9 = spool.tile([P, NT], F32, tag="rd9")
                z_prev = None
                for hop in range(4):
                    t = tpool.tile([P, NT, U], F32, tag="t")
                    if hop == 0:
                        # moving operand carries [z0 | 1]; D lands in col U.
                        # Two 1-bank psum tiles: a 65-col accumulation group
                        # cannot cross a PSUM bank boundary.
                        halves = [
                            dpsp.tile([P, NT // 2, U + 1], F32, tag="hps1",
                                      name=f"hps1_{h}_{i}")
                            for i in (0, 1)
                        ]
                        for nt in range(NT):
                            hp = halves[nt // 4]
                            for mt in range(NT):
                                nc.tensor.matmul(
                                    hp[:, nt % 4, :],
                                    e_sb[:, mt, nt * P:(nt + 1) * P],
                                    v01_sb[:, mt, h, :],
                                    start=(mt == 0), stop=(mt == NT - 1),
                                )
                        rdraw = spool.tile([P, NT], F32, tag="rdraw")
                        nc.vector.reciprocal(rdraw[:, 0:4], halves[0][:, :, U])
                        nc.vector.reciprocal(rdraw[:, 4:8], halves[1][:, :, U])
                        nc.vector.tensor_scalar_mul(rd09[:], rdraw[:], 1.0 - ALPHA)
                        nc.vector.tensor_scalar_mul(rd9[:], rdraw[:],
                                                    (1.0 - ALPHA) / ALPHA)
                        for i in (0, 1):
                            nc.vector.tensor_tensor(
                                t[:, 4 * i:4 * (i + 1), :],
                                halves[i][:, :, 0:U],
                                rd9[:, 4 * i:4 * (i + 1), None].to_broadcast(
                                    [P, 4, U]),
                                mybir.AluOpType.mult,
                            )
                    else:
                        hps = ps512.tile([P, NT, U], F32, tag="ps512")
                        for nt in range(NT):
                            for mt in range(NT):
                                nc.tensor.matmul(
                                    hps[:, nt, :],
                                    e_sb[:, mt, nt * P:(nt + 1) * P],
                                    z_prev[:, mt, :],
                                    start=(mt == 0), stop=(mt == NT - 1),
                                )
                        scale = rd09[:, :, None].to_broadcast([P, NT, U])
                        nc.vector.tensor_tensor(t[:], hps[:], scale,
                                                mybir.AluOpType.mult)
                    if hop == 3:
                        out_ap = y_sb[:, :, h * U:(h + 1) * U]
                    else:
                        znew = zpool.tile([P, NT, U], ZDT, tag="z")
                        out_ap = znew[:]
                    nc.vector.tensor_tensor(out_ap, t[:], w0, mybir.AluOpType.add)
                    if hop != 3:
                        z_prev = znew

            # ---- phase 3: residual + LayerNorm ----
            for nt in range(NT):
                yt = y_sb[:, nt, :]
                nc.vector.tensor_tensor(yt, yt, xn_sb[:, nt, :], mybir.AluOpType.add)
                st6 = spool.tile([P, 6], F32, tag="st6")
                nc.vector.bn_stats(st6[:], yt)
                st2 = spool.tile([P, 2], F32, tag="st2")
                nc.vector.bn_aggr(st2[:], st6[:])
                sd = spool.tile([P, 1], F32, tag="sd")
                nc.scalar.activation(
                    sd[:], st2[:, 1:2], mybir.ActivationFunctionType.Sqrt,
                    bias=eps_sb[:, :],
                )
                rstd = spool.tile([P, 1], F32, tag="rstd")
                nc.vector.reciprocal(rstd[:], sd[:])
                nc.vector.tensor_scalar(
                    yt, yt, st2[:, 0:1], rstd[:],
                    mybir.AluOpType.subtract, mybir.AluOpType.mult,
                )
                if apply_affine:
                    nc.vector.tensor_tensor(yt, yt, gam_sb[:, :], mybir.AluOpType.mult)
                    nc.vector.tensor_tensor(yt, yt, bet_sb[:, :], mybir.AluOpType.add)
                nc.sync.dma_start(
                    out_d[:, :].rearrange("(t p) h -> p t h", p=P)[:, nt, :], yt)

    nc.finalize()
    _BUILD_CACHE[key] = nc
    return nc


def make_in_maps(x, adj, Wq, bq, Wk, bk, Wv, bv, gamma, beta, edt_name, apply_affine):
    import ml_dtypes
    np_edt = np.float32 if edt_name == "float32" else ml_dtypes.bfloat16
    x = np.ascontiguousarray(np.asarray(x, np.float32))
    adj = np.asarray(adj)
    wqT = np.ascontiguousarray(np.asarray(Wq, np.float32).T)
    wkT = np.ascontiguousarray(np.asarray(Wk, np.float32).T)
    wvT01 = np.ascontiguousarray((ALPHA * np.asarray(Wv, np.float32)).T)
    bq = np.asarray(bq, np.float32)
    bk = np.asarray(bk, np.float32)
    bv01 = ALPHA * np.asarray(bv, np.float32)
    in_maps = []
    for b in range(B):
        m = {
            "xT": np.ascontiguousarray(x[b].T),
            "xn": x[b],
            "maskT": np.ascontiguousarray((adj[b] != 0).T.astype(np_edt)),
            "wqT": wqT, "wkT": wkT, "wvT01": wvT01,
            "bq": bq, "bk": bk, "bv01": bv01,
        }
        if apply_affine:
            m["gammar"] = np.ascontiguousarray(
                np.broadcast_to(np.asarray(gamma, np.float32), (P, H)))
            m["betar"] = np.ascontiguousarray(
                np.broadcast_to(np.asarray(beta, np.float32), (P, H)))
        in_maps.append(m)
    return in_maps


def kernel(x, adj, Wq, bq, Wk, bk, Wv, bv, gamma, beta, _trace=False):
    apply_affine = not (
        np.allclose(np.asarray(gamma), 1.0) and np.allclose(np.asarray(beta), 0.0)
    )
    nc = build_nc(E_DTYPE_NAME, apply_affine)
    in_maps = make_in_maps(
        x, adj, Wq, bq, Wk, bk, Wv, bv, gamma, beta, E_DTYPE_NAME, apply_affine
    )
    res = run_bass_kernel_spmd(nc, in_maps, list(range(B)), trace=_trace)
    out = np.stack([np.asarray(res.results[b]["out"]) for b in range(B)])
    if _trace:
        return out.astype(np.float32), res
    return out.astype(np.float32)



# revision 66
# speedup vs baseline: 1.8257x; 1.8257x over previous
"""GAT layer (8-head attention + 4-hop propagation + LayerNorm) on 8 TRN2 cores.

Sharding: data-parallel over batch B=8 — core b computes batch b entirely
(no collectives). Per-core program (all f32 I/O, E/z dtype configurable):

  qT/kT = Wq/Wk @ x^T + b      [512, 1024]  (hidden on partitions)
  v01   = 0.1*(x @ Wv^T + bv)  [1024, 512]  (nodes on partitions)
  per head h (64-dim slices of qT/kT/v01):
    E[m, n] = exp(k_h[m]·q_h[n]) * mask[m, n]          (scores transposed)
    D[n]    = sum_m E[m, n]      (ones-column matmul, fused with hop 1)
    z1 = (E.T @ v01_h) * (9/D)  + v01_h                (= 0.9*P@v + 0.1*v)
    z_{t+1} = (E.T @ z_t) * (0.9/D) + v01_h            (hops 2..4)
    y[:, h*64:+64] = z4
  out = LayerNorm(y + x) * gamma + beta

The softmax max-subtraction is skipped: scores are bounded (~22) so exp()
cannot overflow fp32, and exp(MASK)=0 exactly matches the reference's
masked softmax (verified rel err 5.5e-6 vs jax reference in f32).
"""

import numpy as np

import concourse.bass as bass
import concourse.mybir as mybir
import concourse.tile as tile
from concourse import bacc
from concourse.bass_utils import run_bass_kernel_spmd

B, N, H = 8, 1024, 512
NHEADS, U = 8, 64
P = 128
NT = N // P          # 8 node tiles
KT = H // P          # 4 hidden tiles
ALPHA = 0.1
LN_EPS = 1e-5
F32 = mybir.dt.float32
BF16 = mybir.dt.bfloat16

# E/z dtype for the propagation path: "float32" (rel err ~6e-6) or
# "bfloat16" (rel err ~1.2e-3, ~2x faster weight loads on the hop matmuls).
E_DTYPE_NAME = "bfloat16"

_BUILD_CACHE = {}


def build_nc(edt_name: str, apply_affine: bool):
    key = (edt_name, apply_affine)
    if key in _BUILD_CACHE:
        return _BUILD_CACHE[key]

    EDT = getattr(mybir.dt, edt_name)
    ZDT = EDT
    nc = bacc.Bacc(None, target_bir_lowering=False)

    xT_d = nc.dram_tensor("xT", [H, N], BF16, kind="ExternalInput")
    xn_d = nc.dram_tensor("xn", [N, H], F32, kind="ExternalInput")
    maskT_d = nc.dram_tensor("maskT", [N, N], EDT, kind="ExternalInput")
    wq_d = nc.dram_tensor("wqT", [H, H], BF16, kind="ExternalInput")
    wk_d = nc.dram_tensor("wkT", [H, H], BF16, kind="ExternalInput")
    wv_d = nc.dram_tensor("wvT01", [H, H], BF16, kind="ExternalInput")
    bq_d = nc.dram_tensor("bq", [H], F32, kind="ExternalInput")
    bk_d = nc.dram_tensor("bk", [H], F32, kind="ExternalInput")
    bv_d = nc.dram_tensor("bv01", [H], BF16, kind="ExternalInput")
    if apply_affine:
        gam_d = nc.dram_tensor("gammar", [P, H], F32, kind="ExternalInput")
        bet_d = nc.dram_tensor("betar", [P, H], F32, kind="ExternalInput")
    out_d = nc.dram_tensor("out", [N, H], F32, kind="ExternalOutput")

    with tile.TileContext(nc) as tc:
        with tc.tile_pool(name="const", bufs=1) as cpool, \
             tc.tile_pool(name="big", bufs=1) as bpool, \
             tc.tile_pool(name="epool", bufs=2 if edt_name == "bfloat16" else 1) as epool, \
             tc.tile_pool(name="zpool", bufs=3) as zpool, \
             tc.tile_pool(name="tpool", bufs=2) as tpool, \
             tc.tile_pool(name="spool", bufs=8) as spool, \
             tc.tile_pool(name="wxpool", bufs=2) as wxpool, \
             tc.tile_pool(name="ph1", bufs=1) as p1, \
             tc.tile_pool(name="ps512", bufs=2, space="PSUM") as ps512, \
             tc.tile_pool(name="scps", bufs=2, space="PSUM") as scps, \
             tc.tile_pool(name="dps", bufs=2, space="PSUM") as dpsp:

            # ---- persistent SBUF residents ----
            # Loads spread over the sync/gpsimd/scalar/vector DMA queues so
            # the PE-critical tensors (xT + weights) land first in parallel.
            bq_sb = cpool.tile([P, KT], F32)
            bk_sb = cpool.tile([P, KT], F32)
            bvrow_sb = cpool.tile([P, H], BF16, tag="bvrow")
            maskT_sb = cpool.tile([P, NT, N], EDT)
            xn_sb = cpool.tile([P, NT, H], F32)
            ones_row = cpool.tile([P, P], BF16, tag="onesrow")
            nc.vector.memset(ones_row[:1, :], 1.0)
            eps_sb = cpool.tile([P, 1], F32, tag="eps")
            nc.vector.memset(eps_sb[:], LN_EPS)
            if apply_affine:
                gam_sb = cpool.tile([P, H], F32, tag="gam")
                bet_sb = cpool.tile([P, H], F32, tag="bet")

            qT_sb = bpool.tile([P, KT, N], BF16, tag="qT")
            kT_sb = bpool.tile([P, KT, N], BF16, tag="kT")
            # v01 per-head blocks of 64 values + a trailing 1.0 column; the
            # ones column rides hop 1's moving operand to produce D in PSUM.
            v01_sb = bpool.tile([P, NT, NHEADS, U + 1], ZDT, tag="v01")
            nc.vector.memset(v01_sb[:, :, :, U:U + 1], 1.0)
            y_sb = bpool.tile([P, NT, H], F32, tag="y")
            # LN partial stats per head: head h's stats are interleaved into
            # head h+1's hop loop (2 per hop, in DVE slack); head 7's run in
            # the tail. (walrus: bn_stats out must be exactly [P, 6].)
            st6_sb = bpool.tile([P, NT, NHEADS, 6], F32, tag="st6all")

            # ---- phase 1: projections ----
            if True:
                # xT split into two TILES on two queues: separate tiles keep
                # the first q matmuls off the second half's completion.
                xT_r = xT_d[:, :].rearrange("(t p) n -> p t n", p=P)
                xTa_sb = p1.tile([P, KT, 512], BF16, tag="xTa")
                xTb_sb = p1.tile([P, KT, 512], BF16, tag="xTb")
                nc.sync.dma_start(xTa_sb[:], xT_r[:, :, 0:512])
                nc.sync.dma_start(xTb_sb[:], xT_r[:, :, 512:N])
                wq_sb = p1.tile([P, KT, H], BF16, tag="wq")
                nc.gpsimd.dma_start(wq_sb[:], wq_d[:, :].rearrange("(t p) i -> p t i", p=P))
                wk_sb = p1.tile([P, KT, H], BF16, tag="wk")
                nc.scalar.dma_start(wk_sb[:], wk_d[:, :].rearrange("(t p) i -> p t i", p=P))
                wv_sb = p1.tile([P, KT, H], BF16, tag="wv")
                nc.scalar.dma_start(wv_sb[:], wv_d[:, :].rearrange("(t p) i -> p t i", p=P))
                nc.gpsimd.dma_start(bq_sb[:], bq_d[:].rearrange("(t p) -> p t", p=P))
                nc.gpsimd.dma_start(bk_sb[:], bk_d[:].rearrange("(t p) -> p t", p=P))
                nc.gpsimd.dma_start(bvrow_sb[:1, :],
                                    bv_d[:].rearrange("(a h) -> a h", a=1))
                # mask halves avoid the ACT queue so exp(h0) isn't stuck
                # behind a DMA transfer; ACT queue stays clear after wk/wv.
                maskT_r = maskT_d[:, :].rearrange("(t p) n -> p t n", p=P)
                nc.gpsimd.dma_start(maskT_sb[:, 0:4, :], maskT_r[:, 0:4, :])
                nc.sync.dma_start(maskT_sb[:, 4:8, :], maskT_r[:, 4:8, :])
                nc.gpsimd.dma_start(xn_sb[:], xn_d[:, :].rearrange("(t p) h -> p t h", p=P))
                if apply_affine:
                    nc.gpsimd.dma_start(gam_sb[:], gam_d[:, :])
                    nc.gpsimd.dma_start(bet_sb[:], bet_d[:, :])

                # qT[i, n] = sum_k WqT[k, i] xT[k, n] + bq[i]
                # bf16 matmuls: 1 cyc/row + FWL weight loads on HW.
                def qk_proj(it):
                    for w_sb, b_sb, dst in ((wq_sb, bq_sb, qT_sb),
                                            (wk_sb, bk_sb, kT_sb)):
                        for ncx, x_sb in enumerate((xTa_sb, xTb_sb)):
                            ps = ps512.tile([P, 512], F32, tag="ps512")
                            for kt in range(KT):
                                nc.tensor.matmul(
                                    ps[:],
                                    w_sb[:, kt, it * P:(it + 1) * P],
                                    x_sb[:, kt, :],
                                    start=(kt == 0), stop=(kt == KT - 1),
                                )
                            nc.vector.tensor_scalar_add(
                                dst[:, it, ncx * 512:(ncx + 1) * 512], ps[:],
                                b_sb[:, it:it + 1],
                            )

                # v01[node, j] = sum_k xT[k, node] WvT01[k, j] + bv01[j]
                def xt_slice(nt):
                    if nt < 4:
                        return xTa_sb[:, :, nt * P:(nt + 1) * P]
                    return xTb_sb[:, :, (nt - 4) * P:(nt - 3) * P]

                def v_proj():
                    for nt in range(NT):
                        x_nt = xt_slice(nt)
                        ps = ps512.tile([P, 512], F32, tag="ps512")
                        nc.tensor.matmul(
                            ps[:], ones_row[:1, :P],
                            bvrow_sb[:1, :],
                            start=True, stop=False,
                        )
                        for kt in range(KT):
                            nc.tensor.matmul(
                                ps[:],
                                x_nt[:, kt, :],
                                wv_sb[:, kt, :],
                                start=False, stop=(kt == KT - 1),
                            )
                        nc.vector.tensor_scalar_add(
                            v01_sb[:, nt, :, 0:U],
                            ps[:].rearrange("p (h u) -> p h u", u=U),
                            0.0,
                        )

            # ---- phase 2: per-head attention + propagation ----
            # Software-pipelined: head h+1's scores/exp/mask are ISSUED before
            # head h's hops, so the PE FIFO has ready matmuls (hops of h)
            # while ACT computes exp for h+1. Engine queues are in-order, so
            # program order is what creates cross-head overlap.
            def scores_exp_mask(h, pool_masks=False):
                pt, po = h // 2, (h % 2) * U
                kh = kT_sb[po:po + U, pt, :]   # [64, 1024] (d on partitions)
                qh = qT_sb[po:po + U, pt, :]
                e_sb = epool.tile([P, NT, N], EDT, tag="E", name=f"E_{h}")
                for mt in range(NT):
                    sps = scps.tile([P, N], F32, tag="scps")
                    for ncx in range(2):
                        nc.tensor.matmul(
                            sps[:, ncx * 512:(ncx + 1) * 512],
                            kh[:, mt * P:(mt + 1) * P],
                            qh[:, ncx * 512:(ncx + 1) * 512],
                            start=True, stop=True,
                        )
                    nc.scalar.activation(
                        e_sb[:, mt, :], sps[:], mybir.ActivationFunctionType.Exp,
                    )
                    # mask mult alternates DVE/Pool to balance engine load;
                    # all-Pool when DVE is busy with projection drains (h=0)
                    eng = nc.gpsimd if (pool_masks or mt % 2 == 1) else nc.vector
                    eng.tensor_tensor(
                        e_sb[:, mt, :], e_sb[:, mt, :], maskT_sb[:, mt, :],
                        mybir.AluOpType.mult,
                    )
                return e_sb

            def hops(h, e_sb):
                # Previous head's LN stats (tiny [P,64] calls), deferred to
                # this head's hop loop: 2 per hop lands in DVE slack windows
                # (a block of 8 at once would head-block DVE's hop mults).
                stats_prev = []
                if h >= 1:
                    g = h - 1
                    stats_prev = [
                        lambda nt=nt, g=g: nc.vector.bn_stats(
                            st6_sb[:, nt, g, :],
                            y_sb[:, nt, g * U:(g + 1) * U])
                        for nt in range(NT)
                    ]
                w0 = v01_sb[:, :, h, 0:U]  # [128, 8, 64]
                # hop-3 combines t + (w0 + xn_head) in one add; precompute
                # the sum here so the tail chain is one Pool op shorter.
                w0xn = wxpool.tile([P, NT, U], F32, tag="w0xn", name=f"wx_{h}")
                nc.gpsimd.tensor_tensor(
                    w0xn[:], v01_sb[:, :, h, 0:U],
                    xn_sb[:, :, h * U:(h + 1) * U], mybir.AluOpType.add,
                )
                rd09 = spool.tile([P, NT], F32, tag="rd09")
                rd9 = spool.tile([P, NT], F32, tag="rd9")
                z_prev = None
                for hop in range(4):
                    t = tpool.tile([P, NT, U], F32, tag="t")
                    if hop == 0:
                        # moving operand carries [z0 | 1]; D lands in col U.
                        # Two 1-bank psum tiles: a 65-col accumulation group
                        # cannot cross a PSUM bank boundary.
                        halves = [
                            dpsp.tile([P, NT // 2, U + 1], F32, tag="hps1",
                                      name=f"hps1_{h}_{i}")
                            for i in (0, 1)
                        ]
                        for nt in range(NT):
                            hp = halves[nt // 4]
                            for mt in range(NT):
                                nc.tensor.matmul(
                                    hp[:, nt % 4, :],
                                    e_sb[:, mt, nt * P:(nt + 1) * P],
                                    v01_sb[:, mt, h, :],
                                    start=(mt == 0), stop=(mt == NT - 1),
                                )
                        rdraw = spool.tile([P, NT], F32, tag="rdraw")
                        nc.vector.reciprocal(rdraw[:, 0:4], halves[0][:, :, U])
                        nc.vector.reciprocal(rdraw[:, 4:8], halves[1][:, :, U])
                        nc.vector.tensor_scalar_mul(rd09[:], rdraw[:], 1.0 - ALPHA)
                        nc.vector.tensor_scalar_mul(rd9[:], rdraw[:],
                                                    (1.0 - ALPHA) / ALPHA)
                        for i in (0, 1):
                            nc.vector.tensor_tensor(
                                t[:, 4 * i:4 * (i + 1), :],
                                halves[i][:, :, 0:U],
                                rd9[:, 4 * i:4 * (i + 1), None].to_broadcast(
                                    [P, 4, U]),
                                mybir.AluOpType.mult,
                            )
                    else:
                        hps = ps512.tile([P, NT, U], F32, tag="ps512")
                        for nt in range(NT):
                            for mt in range(NT):
                                nc.tensor.matmul(
                                    hps[:, nt, :],
                                    e_sb[:, mt, nt * P:(nt + 1) * P],
                                    z_prev[:, mt, :],
                                    start=(mt == 0), stop=(mt == NT - 1),
                                )
                        # scale in nt-halves: half A's mult runs while half
                        # B's matmuls still stream -> shorter hop boundary
                        for i in (0, 1):
                            sl = slice(4 * i, 4 * (i + 1))
                            nc.vector.tensor_tensor(
                                t[:, sl, :], hps[:, sl, :],
                                rd09[:, sl, None].to_broadcast([P, 4, U]),
                                mybir.AluOpType.mult)
                    # SBUF-only adds -> Pool engine (DVE is near-saturated),
                    # in nt-halves so downstream consumers start earlier.
                    if hop == 3:
                        # y = t + (w0 + xn_head): residual folded via w0xn
                        for i in (0, 1):
                            sl = slice(4 * i, 4 * (i + 1))
                            nc.gpsimd.tensor_tensor(
                                y_sb[:, sl, h * U:(h + 1) * U], t[:, sl, :],
                                w0xn[:, sl, :], mybir.AluOpType.add,
                            )
                        if h == NHEADS - 1:
                            for nt in range(NT):
                                nc.vector.bn_stats(
                                    st6_sb[:, nt, h, :],
                                    y_sb[:, nt, h * U:(h + 1) * U],
                                )
                    else:
                        znew = zpool.tile([P, NT, U], ZDT, tag="z")
                        for i in (0, 1):
                            sl = slice(4 * i, 4 * (i + 1))
                            nc.gpsimd.tensor_tensor(
                                znew[:, sl, :], t[:, sl, :], w0[:, sl, :],
                                mybir.AluOpType.add)
                        z_prev = znew
                    # 2 deferred stats per hop in DVE's idle window
                    for _ in range(2):
                        if stats_prev:
                            stats_prev.pop(0)()

            # Schedule: q/k for it=0 first, then head 0's scores so exp(h0)
            # (8.3us of ACT) overlaps the rest of phase 1 on PE; then the
            # per-head pipeline (scores h+1 issued before hops h).
            qk_proj(0)
            prev = scores_exp_mask(0, pool_masks=True)
            for it in range(1, KT):
                qk_proj(it)
            v_proj()
            for h in range(1, NHEADS):
                nxt = scores_exp_mask(h)
                hops(h - 1, prev)
                prev = nxt
            hops(NHEADS - 1, prev)

            # ---- phase 3: LayerNorm (residual already folded into y) ----
            # All stats/sqrt/recip first (tiny), then TSPs on two engines,
            # then DMAs on three queues — keeps sqrts from queuing behind
            # 790ns out-DMA transfers on the ACT queue.
            out_r = out_d[:, :].rearrange("(t p) h -> p t h", p=P)
            dma_engs = [nc.sync, nc.scalar, nc.gpsimd]
            st2s, rstds = [], []
            for nt in range(NT):
                st2 = spool.tile([P, 2], F32, tag="st2", name=f"st2_{nt}")
                nc.vector.bn_aggr(st2[:], st6_sb[:, nt, :, :])
                sd = spool.tile([P, 1], F32, tag="sd")
                nc.scalar.activation(
                    sd[:], st2[:, 1:2], mybir.ActivationFunctionType.Sqrt,
                    bias=eps_sb[:, :],
                )
                rstd = spool.tile([P, 1], F32, tag="rstd", name=f"rstd_{nt}")
                nc.vector.reciprocal(rstd[:], sd[:])
                st2s.append(st2)
                rstds.append(rstd)
            for nt in range(NT):
                yt = y_sb[:, nt, :]
                tsp_eng = nc.gpsimd if nt % 2 == 0 else nc.vector
                tsp_eng.tensor_scalar(
                    yt, yt, st2s[nt][:, 0:1], rstds[nt][:],
                    mybir.AluOpType.subtract, mybir.AluOpType.mult,
                )
                if apply_affine:
                    nc.vector.tensor_tensor(yt, yt, gam_sb[:, :], mybir.AluOpType.mult)
                    nc.vector.tensor_tensor(yt, yt, bet_sb[:, :], mybir.AluOpType.add)
            # Pool's DMAs go last so they don't block its TSPs (queue FIFO)
            dma_map = [0, 1, 0, 1, 2, 0, 1, 2]
            for nt in range(NT):
                dma_engs[dma_map[nt]].dma_start(out_r[:, nt, :], y_sb[:, nt, :])

    nc.finalize()
    _BUILD_CACHE[key] = nc
    return nc


def make_in_maps(x, adj, Wq, bq, Wk, bk, Wv, bv, gamma, beta, edt_name, apply_affine):
    import ml_dtypes
    bf16 = ml_dtypes.bfloat16
    np_edt = np.float32 if edt_name == "float32" else bf16
    x = np.ascontiguousarray(np.asarray(x, np.float32))
    adj = np.asarray(adj)
    wqT = np.ascontiguousarray(np.asarray(Wq, np.float32).T.astype(bf16))
    wkT = np.ascontiguousarray(np.asarray(Wk, np.float32).T.astype(bf16))
    wvT01 = np.ascontiguousarray((ALPHA * np.asarray(Wv, np.float32)).T.astype(bf16))
    bq = np.asarray(bq, np.float32)
    bk = np.asarray(bk, np.float32)
    bv01 = (ALPHA * np.asarray(bv, np.float32)).astype(bf16)
    in_maps = []
    for b in range(B):
        m = {
            "xT": np.ascontiguousarray(x[b].T.astype(bf16)),
            "xn": x[b],
            "maskT": np.ascontiguousarray((adj[b] != 0).T.astype(np_edt)),
            "wqT": wqT, "wkT": wkT, "wvT01": wvT01,
            "bq": bq, "bk": bk, "bv01": bv01,
        }
        if apply_affine:
            m["gammar"] = np.ascontiguousarray(
                np.broadcast_to(np.asarray(gamma, np.float32), (P, H)))
            m["betar"] = np.ascontiguousarray(
                np.broadcast_to(np.asarray(beta, np.float32), (P, H)))
        in_maps.append(m)
    return in_maps


def kernel(x, adj, Wq, bq, Wk, bk, Wv, bv, gamma, beta, _trace=False):
    apply_affine = not (
        np.allclose(np.asarray(gamma), 1.0) and np.allclose(np.asarray(beta), 0.0)
    )
    nc = build_nc(E_DTYPE_NAME, apply_affine)
    in_maps = make_in_maps(
        x, adj, Wq, bq, Wk, bk, Wv, bv, gamma, beta, E_DTYPE_NAME, apply_affine
    )
    res = run_bass_kernel_spmd(nc, in_maps, list(range(B)), trace=_trace)
    out = np.stack([np.asarray(res.results[b]["out"]) for b in range(B)])
    if _trace:
        return out.astype(np.float32), res
    return out.astype(np.float32)

